# revision 1
# baseline (speedup 1.0000x reference)
"""Tensor-parallel fake-quant attention on 8 Trainium2 NeuronCores.

Sharding (per sharding hint): tensor-parallel over heads.
  - 16 q-heads -> 2 per core (Wq/bq row-sharded, 256 rows each).
  - 4 kv-heads -> kv-head d//2 on core d (each kv head redundantly
    computed on the 2 cores that own its query group), so the RoPE/cache
    chain needs no cross-core data exchange.
  - KV cache sharded by kv-head.
  - Wo column-sharded (256 cols per core); o_proj partials are summed
    across cores at the end (the all-reduce of the hint).
Fake-quant scales are global amaxes in the reference, so every fq uses a
cross-core max (lax.pmax) over the per-core shard amax; shard unions
cover each full tensor, which reproduces the reference scale exactly.
"""

import numpy as np

S, HID, H, KVH, D, C = 1024, 2048, 16, 4, 128, 4096
NCORES = 8
HPC = H // NCORES          # q-heads per core = 2
ROWS_Q = HPC * D           # 256 Wq rows per core

_jax_state = {}


def _get_jax():
    if _jax_state:
        return _jax_state
    import jax
    jax.config.update("jax_default_matmul_precision", "highest")
    import jax.numpy as jnp
    from jax import lax
    _jax_state.update(jax=jax, jnp=jnp, lax=lax)
    return _jax_state


def _device_fn(axis_name):
    st = _get_jax()
    jnp, lax = st["jnp"], st["lax"]

    def fq(x, bits):
        qmax = 2.0 ** (bits - 1) - 1.0
        amax = jnp.max(jnp.abs(x))
        if axis_name is not None:
            amax = lax.pmax(amax, axis_name)
        scale = jnp.maximum(amax, 1e-8) / qmax
        return jnp.clip(jnp.round(x / scale), -qmax - 1.0, qmax) * scale

    def rot_half(t, half):
        t = fq(t, 16)
        t1, t2 = t[..., :half], t[..., half:]
        t2 = fq(-t2, 16)
        return fq(jnp.concatenate([t2, t1], axis=-1), 16)

    def f(x, cos, sin, mask, cache_k_d, cache_v_d,
          Wq_d, bq_d, Wk_d, bk_d, Wv_d, bv_d, Wo_d):
        import jax as _jax
        xq = fq(x, 8)
        q = xq @ fq(Wq_d, 8).T + bq_d          # [S, 256]
        k = xq @ fq(Wk_d, 8).T + bk_d          # [S, 128]
        v = xq @ fq(Wv_d, 8).T + bv_d          # [S, 128]
        k_out, v_out = k, v

        q = q.reshape(S, HPC, D).transpose(1, 0, 2)   # [2, S, D]
        kh = k[None]                                   # [1, S, D]
        q16 = fq(q, 16)
        k16 = fq(kh, 16)
        cos_b = cos[None]
        sin_b = sin[None]
        q_emb = fq(fq(q16 * cos_b, 16)
                   + fq(rot_half(q16, D // 2) * sin_b, 16), 16)
        k_emb = (fq(k16 * cos_b, 16)
                 + fq(rot_half(k16, D // 2) * sin_b, 16))  # no outer fq

        ck = jnp.concatenate([cache_k_d[S:], k_emb[0]], axis=0)  # [C, D]
        cv = jnp.concatenate([cache_v_d[S:], v], axis=0)         # [C, D]

        attn = jnp.einsum('hsd,cd->hsc', fq(q_emb, 8), fq(ck, 16))
        attn = attn * (1.0 / np.sqrt(D))
        attn = fq(attn + mask[None], 16)
        attn = fq(_jax.nn.softmax(attn, axis=-1), 16)
        out = jnp.einsum('hsc,cd->hsd', attn, fq(cv, 8))  # [2, S, D]

        a = out.transpose(1, 0, 2).reshape(S, HPC * D)    # [S, 256]
        partial = fq(a, 8) @ fq(Wo_d, 8).T                # [S, HID]
        return partial, k_out, v_out

    return f


def _shard_inputs(hidden_states, cos, sin, cache_k, cache_v, mask,
                  Wq, bq, Wk, bk, Wv, bv, Wo):
    x = np.ascontiguousarray(hidden_states[0])          # [S, HID]
    cos2 = np.ascontiguousarray(cos[0])                 # [S, D]
    sin2 = np.ascontiguousarray(sin[0])
    rep = lambda a: np.broadcast_to(a, (NCORES,) + a.shape)
    kv_idx = [d // 2 for d in range(NCORES)]
    args = (
        rep(x), rep(cos2), rep(sin2), rep(mask),
        np.stack([cache_k[g] for g in kv_idx]),
        np.stack([cache_v[g] for g in kv_idx]),
        Wq.reshape(NCORES, ROWS_Q, HID),
        bq.reshape(NCORES, ROWS_Q),
        np.stack([Wk[D * g:D * (g + 1)] for g in kv_idx]),
        np.stack([bk[D * g:D * (g + 1)] for g in kv_idx]),
        np.stack([Wv[D * g:D * (g + 1)] for g in kv_idx]),
        np.stack([bv[D * g:D * (g + 1)] for g in kv_idx]),
        np.stack([np.ascontiguousarray(Wo[:, ROWS_Q * d:ROWS_Q * (d + 1)])
                  for d in range(NCORES)]),
    )
    return tuple(np.ascontiguousarray(a, dtype=np.float32) for a in args)


def _assemble(partial, k_o, v_o):
    out = np.sum(np.asarray(partial, dtype=np.float32), axis=0)[None]
    k_out = np.stack([np.asarray(k_o[2 * g]) for g in range(KVH)], axis=1)
    v_out = np.stack([np.asarray(v_o[2 * g]) for g in range(KVH)], axis=1)
    return out, k_out, v_out


_cache = {}


def kernel(hidden_states, cos, sin, cache_k, cache_v, mask,
           Wq, bq, Wk, bk, Wv, bv, Wo):
    st = _get_jax()
    jax = st["jax"]
    args = _shard_inputs(hidden_states, cos, sin, cache_k, cache_v, mask,
                         Wq, bq, Wk, bk, Wv, bv, Wo)

    if "mode" not in _cache:
        devs = jax.devices()[:NCORES]
        try:
            fp = jax.pmap(_device_fn("i"), axis_name="i", devices=devs)
            res = fp(*args)
            res = tuple(np.asarray(r) for r in res)
            _cache["mode"] = ("pmap", fp)
            return _assemble(*res)
        except Exception:
            # Collectives unavailable: run each core's shard without pmax
            # using host-combined scales is not equivalent; fall back to
            # a faithful single-device run.
            f1 = jax.jit(_single_device_fn())
            _cache["mode"] = ("single", f1)

    mode, fn = _cache["mode"]
    if mode == "pmap":
        res = fn(*args)
        return _assemble(*(np.asarray(r) for r in res))
    full = (hidden_states, cos, sin, cache_k, cache_v, mask,
            Wq, bq, Wk, bk, Wv, bv, Wo)
    out, k_out, v_out = fn(*[np.asarray(a, np.float32) for a in full])
    return np.asarray(out), np.asarray(k_out), np.asarray(v_out)


def _single_device_fn():
    st = _get_jax()
    jnp = st["jnp"]
    import jax as _jax

    def fq(x, bits):
        qmax = 2.0 ** (bits - 1) - 1.0
        scale = jnp.maximum(jnp.max(jnp.abs(x)), 1e-8) / qmax
        return jnp.clip(jnp.round(x / scale), -qmax - 1.0, qmax) * scale

    def rot_half(t, half):
        t = fq(t, 16)
        t1, t2 = t[..., :half], t[..., half:]
        t2 = fq(-t2, 16)
        return fq(jnp.concatenate([t2, t1], axis=-1), 16)

    def f(hidden_states, cos, sin, cache_k, cache_v, mask,
          Wq, bq, Wk, bk, Wv, bv, Wo):
        G = H // KVH
        x = hidden_states[0]
        xq = fq(x, 8)
        q = (xq @ fq(Wq, 8).T + bq).reshape(S, H, D).transpose(1, 0, 2)
        k = xq @ fq(Wk, 8).T + bk
        v = xq @ fq(Wv, 8).T + bv
        k_out = k.reshape(S, KVH, D)
        v_out = v.reshape(S, KVH, D)
        kh = k_out.transpose(1, 0, 2)
        vh = v_out.transpose(1, 0, 2)
        q16 = fq(q, 16)
        k16 = fq(kh, 16)
        q_emb = fq(fq(q16 * cos, 16) + fq(rot_half(q16, D // 2) * sin, 16), 16)
        k_emb = fq(k16 * cos, 16) + fq(rot_half(k16, D // 2) * sin, 16)
        ck = jnp.concatenate([cache_k[:, S:, :], k_emb], axis=1)
        cv = jnp.concatenate([cache_v[:, S:, :], vh], axis=1)
        qr = q_emb.reshape(KVH, G * S, D)
        attn = jnp.einsum('hqd,hkd->hqk', fq(qr, 8), fq(ck, 16))
        attn = attn.reshape(H, S, C) * (1.0 / np.sqrt(D))
        attn = fq(attn + mask, 16)
        attn = fq(_jax.nn.softmax(attn, axis=-1), 16)
        attn = attn.reshape(KVH, G * S, C)
        out = jnp.einsum('hqk,hkd->hqd', attn, fq(cv, 8))
        out = out.reshape(H, S, D).transpose(1, 0, 2).reshape(1, S, H * D)
        out = fq(out, 8) @ fq(Wo, 8).T
        return out, k_out, v_out

    return f
